# revision 59
# baseline (speedup 1.0000x reference)
"""Causal self-attention (B=2, T=2048, C=1024, H=16, D=64) on 8 trn2 cores.

Sharding: core c handles batch b = c//4 and head group hg = c%4 (heads
4*hg .. 4*hg+3).  Each core computes q/k/v projections for its 4 heads,
causal-softmax attention, and a partial output projection
y_partial = O_heads @ Wo[:, heads].T.  The host sums the 4 partials per
batch and adds the bias.

Numerics (verified ~9e-3 max-rel vs the 2e-2 gate, fp32 ref ~5e-7):
  - q/k projections run in fp8e4m3 with perf_mode=DoubleRow (2 weights
    per PE cell, K=256 per matmul): weights pre-scaled by 16 (folded
    back out through the exp scale), activations straight-cast.
  - everything else (V/output projections, S, P, O) uses bf16 matmul
    operands with fp32 PSUM accumulation; y partials return in bf16.

Layout:
  qT/kT per pair: 4 chunk tiles [128, 512] bf16 (rows 0:64 head-even,
          64:128 head-odd), copied from the projection PSUM
  V is projected PRE-TRANSPOSED (lhsT = xT tile, rhs = wv tile) into
          [128 Tk, 256] PSUM, then one strided copy per Tk tile into
          vaug[t] [128, 260] = [Vh0|1|Vh1|1|Vh2|1|Vh3|1]; the ones
          columns (memset once) make PSUM row 64 of the O^T
          accumulation the softmax denominator
  S^T_j = kT[j].T @ qT[I]  (k-major, K=64 per head, causally trimmed)
  P = exp(S/(sqrt(C)*256)) on ACT, batched [128,1024] per head-group,
          then the boundary strip is masked on DVE
  O^T_h = sum_j vaug[j][:, 65h:65h+65].T @ P_j  -> [65, 512] PSUM
  normalize: reciprocal (DVE) -> partition_broadcast (GPSIMD) -> mul;
          head-odd half shifted to partitions 64:128 via SBUF DMA
  y: per (Tk, 512-col) unit, 2 matmuls (K=256 over the head pairs) ->
          bf16 copy into a persistent per-chunk staging tile -> one
          512-row DMA per half chunk (large DMAs amortize the ~2.2us
          per-DMA completion serialization on the SP queue)

Emission / scheduling: the Tile scheduler is a readiness+priority list
scheduler; the per-engine streams it produces execute in order.  The
attention inner chain (S -> exp -> mask -> O) is ACT-bound, so the
emission interleaves next-chunk projection groups and y-projection
units into the attention stream as PE filler, paced by an ACT-vs-PE
deficit tracker (pay_debt).  The last chunk holds back a few y units
as tail warmers so the final normalize chain overlaps PE work.  All
DMAs are batched (one per weight matrix / x chunk / half y chunk) --
each DMA instruction costs ~650ns HWDGE issue plus ~1.5us completion
serialization on its queue.
"""
import numpy as np
import ml_dtypes

import concourse.tile as tile
import concourse.mybir as mybir
from concourse import bacc
from concourse.bass_utils import run_bass_kernel_spmd

FP = mybir.dt.float32
BF = mybir.dt.bfloat16
F8 = mybir.dt.float8e4
W8SCALE = 16.0  # fp8 weight pre-scale (folded back out via the exp scale)
NDT = 4  # DoubleRow K-tiles over the embedding dim (4 x (128x2))
B, T, C = 2, 2048, 1024
H, D = 16, 64
SCALE = 1.0 / 32.0  # 1/sqrt(C)
S_SCALE = SCALE / (W8SCALE * W8SCALE)  # q,k carry a 16x pre-scale each
N_CORES = 8
NKT = C // 128  # 8 K-tiles over the embedding dim
NTK = T // 128  # 16 Tk tiles
NI = T // 512  # 4 Tq chunks
EXP = mybir.ActivationFunctionType.Exp

_nc_cache = {}


def build_kernel(repeats=1, hmix=False):
    key = (repeats, hmix)
    if key in _nc_cache:
        return _nc_cache[key]

    nc = bacc.Bacc("TRN2", target_bir_lowering=False, debug=False)

    xT_d = nc.dram_tensor("xT", [C, T], BF, kind="ExternalInput").ap()
    xq8_d = nc.dram_tensor("xq8", [128, NDT, 2, T], F8, kind="ExternalInput").ap()
    wq8_d = nc.dram_tensor("wq8", [128, NDT, 2, 256], F8, kind="ExternalInput").ap()
    wk8_d = nc.dram_tensor("wk8", [128, NDT, 2, 256], F8, kind="ExternalInput").ap()
    wvT_d = nc.dram_tensor("wvT", [C, 256], BF, kind="ExternalInput").ap()
    woT_d = nc.dram_tensor("woT", [256, C], BF, kind="ExternalInput").ap()
    y_d = nc.dram_tensor("y", [T, C], BF, kind="ExternalOutput").ap()

    # mask[p, i] = 1 iff i >= p : triangular strip at the causal boundary
    mask_np = (
        np.arange(128)[None, :] >= np.arange(128)[:, None]
    ).astype(ml_dtypes.bfloat16)
    mask_d = nc.inline_tensor(mask_np, "mask_tri").ap()

    with tile.TileContext(nc) as tc:
        with (
            tc.tile_pool(name="persist", bufs=1) as pp,
            tc.tile_pool(name="ppool", bufs=4) as ppool,
            tc.tile_pool(name="spool", bufs=8) as spool,
            tc.tile_pool(name="ps_s", bufs=2, space="PSUM") as ps_s,
            tc.tile_pool(name="ps_o", bufs=2, space="PSUM") as ps_o,
            tc.tile_pool(name="ps_mm", bufs=2, space="PSUM") as ps_mm,
        ):
            # ---- critical-path DMAs first: the fp8 q/k operands are tiny,
            # so the first projection matmuls start in ~2-3us ----
            wq8 = pp.tile([128, NDT, 2, 256], F8, tag="wq8")
            nc.sync.dma_start(wq8[:, :, :, :], wq8_d[:, :, :, :])
            xq8 = [
                pp.tile([128, NDT, 2, 512], F8, tag=f"xq8{c}", name=f"xq8_{c}")
                for c in range(NI)
            ]
            nc.sync.dma_start(xq8[0][:, :, :, :], xq8_d[:, :, :, 0:512])
            wk8 = pp.tile([128, NDT, 2, 256], F8, tag="wk8")
            nc.sync.dma_start(wk8[:, :, :, :], wk8_d[:, :, :, :])
            # xT (bf16) feeds only the V projection; one DMA per chunk
            xT_r = xT_d.rearrange("(n p) t -> p n t", p=128)
            xc = [
                pp.tile([128, NKT, 512], BF, tag=f"xc{c}", name=f"xc{c}")
                for c in range(NI)
            ]
            nc.sync.dma_start(xc[0][:, :, :], xT_r[:, :, 0:512])
            wv_big = pp.tile([128, NKT, 256], BF, tag="wv")
            nc.sync.dma_start(
                wv_big[:, :, :], wvT_d.rearrange("(n p) d -> p n d", p=128)
            )
            for c in range(1, NI):
                nc.sync.dma_start(
                    xq8[c][:, :, :, :], xq8_d[:, :, :, c * 512 : c * 512 + 512]
                )
                nc.sync.dma_start(
                    xc[c][:, :, :], xT_r[:, :, c * 512 : c * 512 + 512]
                )
            wo_big = pp.tile([128, 2, C], BF, tag="wo")
            nc.sync.dma_start(
                wo_big[:, :, :], woT_d.rearrange("(n p) d -> p n d", p=128)
            )
            mask = pp.tile([128, 128], BF, tag="mask")
            nc.sync.dma_start(mask[:], mask_d[:])

            # preload the exp table while DMAs stream (saves ~1.3us later)
            warm_in = pp.tile([1, 2], FP, tag="warm_in")
            warm_out = pp.tile([1, 2], FP, tag="warm_out")
            nc.vector.memset(warm_in[:], 0.0)
            nc.scalar.activation(warm_out[:], warm_in[:], EXP, scale=SCALE)


            # ---- persistent activations ----
            qTc = [
                [pp.tile([128, 512], BF, tag=f"qT{p}_{i}", name=f"qT{p}_{i}") for i in range(NI)]
                for p in range(2)
            ]
            kTc = [
                [pp.tile([128, 512], BF, tag=f"kT{p}_{i}", name=f"kT{p}_{i}") for i in range(NI)]
                for p in range(2)
            ]
            otstc = [
                [pp.tile([128, 512], BF, tag=f"ot{p}_{i}", name=f"ot{p}_{i}") for i in range(NI)]
                for p in range(2)
            ]
            # vaug[t] = [Vh0|1|Vh1|1|Vh2|1|Vh3|1]; ones set once, V cols
            # rewritten per repeat by the strided copy from the V psum
            vaug = [pp.tile([128, 260], BF, tag=f"va{t}", name=f"va{t}") for t in range(NTK)]
            for t in range(NTK):
                nc.vector.memset(vaug[t][:, 64:260:65], 1.0)

            R = [0]

            # ---- PE filler machinery: queues of (cost_ns, emit_fn).
            # proj fillers must all land before the next attention chunk;
            # yproj fillers may linger until the final drain ----
            fillers_proj = []
            fillers_y = []
            debt = [0.0]
            y_reserve = [0]

            def pay_debt():
                while debt[0] > 200.0 and (
                    fillers_proj or len(fillers_y) > y_reserve[0]
                ):
                    q = fillers_proj if fillers_proj else fillers_y
                    cost, fn = q.pop(0)
                    fn()
                    debt[0] -= cost

            def drain_proj_fillers():
                while fillers_proj:
                    _, fn = fillers_proj.pop(0)
                    fn()
                debt[0] = 0.0

            def drain_y_fillers(on_act=False):
                while fillers_y:
                    _, fn = fillers_y.pop(0)
                    fn(on_act=on_act)

            def drain_all_fillers():
                drain_proj_fillers()
                drain_y_fillers(on_act="tail")

            # ---- emission units ----
            def emit_qk_group(c, wts, dsts, nm, pair):
                ps = ps_mm.tile([128, 512], FP, tag="mm",
                                name=f"ps{nm}{c}_{pair}_r{R[0]}")
                for dt in range(NDT):
                    nc.tensor.matmul(
                        ps[:],
                        lhsT=wts[:, dt, :, pair * 128 : pair * 128 + 128],
                        rhs=xq8[c][:, dt, :, :],
                        start=(dt == 0),
                        stop=(dt == NDT - 1),
                        perf_mode=mybir.MatmulPerfMode.DoubleRow,
                    )
                # this copy gates the next chunk's whole attention stream;
                # schedule it ahead of other queued DVE work
                with tc.high_priority(60):
                    nc.vector.tensor_copy(dsts[pair][c][:], ps[:])

            def emit_v_group(t):
                c, ts = t // 4, t % 4
                ps = ps_mm.tile([128, 512], FP, tag="mm",
                                name=f"psv{t}_r{R[0]}")
                for kk in range(NKT):
                    nc.tensor.matmul(
                        ps[:, 0:256],
                        lhsT=xc[c][:, kk, ts * 128 : ts * 128 + 128],
                        rhs=wv_big[:, kk, :],
                        start=(kk == 0),
                        stop=(kk == NKT - 1),
                    )
                with tc.high_priority(60):
                    nc.vector.tensor_copy(
                        vaug[t][:].rearrange("p (n d) -> p n d", n=4)[:, :, 0:64],
                        ps[:, 0:256].rearrange("p (n d) -> p n d", n=4),
                    )

            def emit_proj_chunk(c):
                for pair in range(2):
                    emit_qk_group(c, wq8, qTc, "q", pair)
                for pair in range(2):
                    emit_qk_group(c, wk8, kTc, "k", pair)
                for t in range(4 * c, 4 * c + 4):
                    emit_v_group(t)

            def proj_chunk_fillers(c):
                for pair in range(2):
                    fillers_proj.append((430, lambda pair=pair: emit_qk_group(
                        c, wq8, qTc, "q", pair)))
                for pair in range(2):
                    fillers_proj.append((430, lambda pair=pair: emit_qk_group(
                        c, wk8, kTc, "k", pair)))
                for t in range(4 * c, 4 * c + 4):
                    fillers_proj.append((870, lambda t=t: emit_v_group(t)))

            yt_chunk = [
                pp.tile([128, 4, 1024], BF, tag=f"yc{c}", name=f"yc{c}")
                for c in range(NI)
            ]
            y_done = {}

            def emit_yproj_unit(t, nch, on_act=False):
                ps = ps_mm.tile([128, 512], FP, tag="mm",
                                name=f"psy{t}_{nch}_r{R[0]}")
                for pair in range(2):
                    nc.tensor.matmul(
                        ps[:],
                        lhsT=otstc[pair][t // 4][
                            :, (t % 4) * 128 : (t % 4) * 128 + 128
                        ],
                        rhs=wo_big[:, pair, nch * 512 : nch * 512 + 512],
                        start=(pair == 0),
                        stop=(pair == 1),
                    )
                c = t // 4
                dst = yt_chunk[c][:, t % 4, nch * 512 : nch * 512 + 512]
                # mid-run copies go to DVE (ACT is exp-saturated); in the
                # reserve drain the DVE queue is full of normalize work so
                # use ACT; at the tail both are idle, so alternate
                if on_act == "act" or (
                    on_act == "tail" and (2 * t + nch) % 2 == 0
                ):
                    nc.scalar.copy(dst, ps[:])
                else:
                    nc.vector.tensor_copy(dst, ps[:])
                y_done[c] = y_done.get(c, 0) + 1
                if y_done[c] in (4, 8):  # half-chunk complete -> DMA it
                    lo = 0 if y_done[c] == 4 else 2
                    nc.sync.dma_start(
                        y_d[c * 512 + lo * 128 : c * 512 + lo * 128 + 256, :]
                        .rearrange("(n p) d -> p n d", p=128),
                        yt_chunk[c][:, lo : lo + 2, :],
                    )
                    if y_done[c] == 8:
                        y_done[c] = 0

            def emit_attention(I, last=False, pairs=(0, 1)):
                emit_attention_body(I, last, pairs)
                if 1 in pairs:
                    for t in range(4 * I, 4 * I + 4):
                        for nch in range(2):
                            fillers_y.append(
                                (430,
                                 lambda t=t, nch=nch, **kw: emit_yproj_unit(
                                     t, nch, **kw))
                            )

            def emit_attention_body(I, last, pairs):
                jmax = 4 * I + 4
                for pair in pairs:
                    oT = [
                        ps_o.tile([65, 512], FP, tag="oT",
                                  name=f"o{I}_{pair}_{h}_r{R[0]}")
                        for h in range(2)
                    ]

                    def emit_o(g, tiles):
                        j0 = 2 * g
                        for h in range(2):
                            p_sb = tiles[h]
                            head = 2 * pair + h
                            for dj in range(2):
                                j = j0 + dj
                                z = max(0, j * 128 - I * 512)
                                nc.tensor.matmul(
                                    oT[h][:, z:512],
                                    lhsT=vaug[j][:, 65 * head : 65 * head + 65],
                                    rhs=p_sb[:, dj * 512 + z : dj * 512 + 512],
                                    start=(j == 0),
                                    stop=(j == jmax - 1),
                                )

                    prev = None
                    for g in range(jmax // 2):
                        j0 = 2 * g
                        diag = j0 >= 4 * I
                        zs = [max(0, (j0 + dj) * 128 - I * 512) for dj in range(2)]
                        cur = []
                        act_ns = 0.0
                        cols = 0
                        for h in range(2):
                            hsl = slice(64 * h, 64 * h + 64)
                            s_ps = ps_s.tile([128, 1024], FP, tag="s",
                                             name=f"s{I}_{pair}_{h}_{g}_r{R[0]}")
                            for dj in range(2):
                                j = j0 + dj
                                z = zs[dj]
                                nc.tensor.matmul(
                                    s_ps[:, dj * 512 + z : dj * 512 + 512],
                                    lhsT=kTc[pair][j // 4][
                                        hsl, (j % 4) * 128 : (j % 4) * 128 + 128
                                    ],
                                    rhs=qTc[pair][I][hsl, z:512],
                                    start=True,
                                    stop=True,
                                )
                                cols += 512 - z
                            p_sb = ppool.tile([128, 1024], BF, tag="p",
                                              name=f"p{I}_{pair}_{h}_{g}_r{R[0]}")
                            if not diag or zs[0] == 0:
                                # diag group with z0=0: one act over the whole
                                # tile; the uncomputed gap [512:512+z1] holds
                                # stale S values, bounded so exp stays finite,
                                # and nothing downstream reads it
                                nc.scalar.activation(p_sb[:], s_ps[:], EXP,
                                                     scale=S_SCALE)
                                act_ns += 1024 * 0.833 + 185
                            else:
                                for dj in range(2):
                                    lo = dj * 512 + zs[dj]
                                    hi = dj * 512 + 512
                                    nc.scalar.activation(
                                        p_sb[:, lo:hi], s_ps[:, lo:hi], EXP,
                                        scale=S_SCALE,
                                    )
                                    act_ns += (hi - lo) * 0.833 + 185
                            for dj in range(2):
                                j = j0 + dj
                                if j >= 4 * I:
                                    z = zs[dj]
                                    ssl = slice(dj * 512 + z, dj * 512 + z + 128)
                                    nc.vector.tensor_mul(
                                        p_sb[:, ssl], p_sb[:, ssl], mask[:]
                                    )
                            cur.append(p_sb)
                        # ACT-vs-PE deficit for this group: exp time vs the
                        # S+O matmul time (2x cols at 0.4167ns/col)
                        debt[0] += act_ns - 2 * cols * 0.4167
                        pay_debt()
                        if prev is not None:
                            emit_o(g - 1, prev)
                        prev = cur
                    emit_o(jmax // 2 - 1, prev)
                    if last and pair == 1:
                        # reserved y units keep PE warm through the final
                        # normalize chain
                        y_reserve[0] = 0
                        drain_y_fillers(on_act="act")
                    # normalize O^T by the PSUM row-64 denominator
                    for h in range(2):
                        recip = spool.tile([1, 512], FP, tag="recip",
                                           name=f"rc{I}_{pair}_{h}_r{R[0]}")
                        nc.vector.reciprocal(recip[:], oT[h][64:65, :])
                        bcast = spool.tile([64, 512], FP, tag="bcast",
                                           name=f"bc{I}_{pair}_{h}_r{R[0]}")
                        nc.gpsimd.partition_broadcast(bcast[:], recip[:])
                        if h == 0:
                            nc.vector.tensor_mul(
                                otstc[pair][I][0:64, :], oT[h][0:64, :], bcast[:]
                            )
                        else:
                            onrm = spool.tile([64, 512], BF, tag="onrm",
                                              name=f"on{I}_{pair}_r{R[0]}")
                            nc.vector.tensor_mul(onrm[:], oT[h][0:64, :], bcast[:])
                            # partition shift 0->64 needs a DMA
                            nc.sync.dma_start(otstc[pair][I][64:128, :], onrm[:])


            # ---- main emission.  Attention instructions carry high
            # scheduler priority (they form the serial latency chain:
            # S -> exp -> mask -> O -> normalize); projections and
            # y-projection units are normal priority, so the greedy
            # scheduler slots them into PE whenever attention work is
            # not ready ----
            for rep in range(repeats):
                R[0] = rep
                emit_proj_chunk(0)
                for c in range(NI):
                    if c + 1 < NI:
                        proj_chunk_fillers(c + 1)
                    else:
                        y_reserve[0] = 6
                    emit_attention(c, last=(c == NI - 1))
                    drain_proj_fillers()
                y_reserve[0] = 0
                drain_all_fillers()

    nc.compile()
    _nc_cache[key] = nc
    return nc


def make_in_maps(x, Wq, Wk, Wv, Wo):
    x = np.asarray(x, dtype=np.float32)
    Wq = np.asarray(Wq, dtype=np.float32)
    Wk = np.asarray(Wk, dtype=np.float32)
    Wv = np.asarray(Wv, dtype=np.float32)
    Wo = np.asarray(Wo, dtype=np.float32)
    bf = ml_dtypes.bfloat16
    f8 = ml_dtypes.float8_e4m3fn

    def dr_pack(a):  # [C, m] -> [128, NDT, 2, m] with k = 256*dt + ki + 128*ko
        return np.ascontiguousarray(
            a.reshape(4, 2, 128, -1).transpose(2, 0, 1, 3)
        )

    in_maps = []
    for c in range(N_CORES):
        b, hg = c // 4, c % 4
        sl = slice(256 * hg, 256 * hg + 256)
        xTb = x[b].T
        in_maps.append(
            {
                "xT": np.ascontiguousarray(xTb).astype(bf),
                "xq8": dr_pack(xTb.astype(f8)),
                "wq8": dr_pack((Wq[sl, :].T * W8SCALE).astype(f8)),
                "wk8": dr_pack((Wk[sl, :].T * W8SCALE).astype(f8)),
                "wvT": np.ascontiguousarray(Wv[sl, :].T).astype(bf),
                "woT": np.ascontiguousarray(Wo[:, sl].T).astype(bf),
            }
        )
    return in_maps


def run_spmd(in_maps, trace=False, repeats=1, **kw):
    nc = build_kernel(repeats)
    return run_bass_kernel_spmd(nc, in_maps, list(range(N_CORES)), trace=trace, **kw)


def gather(results, bo):
    bo = np.asarray(bo, dtype=np.float32)
    y = np.empty((B, T, C), dtype=np.float32)
    for b in range(B):
        acc = results[4 * b]["y"].astype(np.float32).copy()
        for g in range(1, 4):
            acc += results[4 * b + g]["y"].astype(np.float32)
        y[b] = acc + bo[None, :]
    return y


def kernel(x, Wq, Wk, Wv, Wo, bo):
    res = run_spmd(make_in_maps(x, Wq, Wk, Wv, Wo))
    return gather(res.results, bo)


# revision 67
# speedup vs baseline: 1.0750x; 1.0750x over previous
"""Causal self-attention (B=2, T=2048, C=1024, H=16, D=64) on 8 trn2 cores.

Sharding: core c handles batch b = c//4 and head group hg = c%4 (heads
4*hg .. 4*hg+3).  Each core computes q/k/v projections for its 4 heads,
causal-softmax attention, and a partial output projection
y_partial = O_heads @ Wo[:, heads].T.  The host sums the 4 partials per
batch and adds the bias.

Numerics (verified ~9e-3 max-rel vs the 2e-2 gate, fp32 ref ~5e-7):
  - q/k projections run in fp8e4m3 with perf_mode=DoubleRow (2 weights
    per PE cell, K=256 per matmul): weights pre-scaled by 16 (folded
    back out through the exp scale), activations straight-cast.
  - everything else (V/output projections, S, P, O) uses bf16 matmul
    operands with fp32 PSUM accumulation; y partials return in bf16.

Layout:
  qT/kT per pair: 4 chunk tiles [128, 512] bf16 (rows 0:64 head-even,
          64:128 head-odd), copied from the projection PSUM
  V is projected PRE-TRANSPOSED (lhsT = xT tile, rhs = wv tile) into
          [128 Tk, 256] PSUM, then one strided copy per Tk tile into
          vaug[t] [128, 260] = [Vh0|1|Vh1|1|Vh2|1|Vh3|1]; the ones
          columns (memset once) make PSUM row 64 of the O^T
          accumulation the softmax denominator
  S^T_j = kT[j].T @ qT[I]  (k-major, K=64 per head, causally trimmed)
  P = exp(S/(sqrt(C)*256)) on ACT, batched [128,1024] per head-group,
          then the boundary strip is masked on DVE
  O^T_h = sum_j vaug[j][:, 65h:65h+65].T @ P_j  -> [65, 512] PSUM
  normalize: reciprocal (DVE) -> partition_broadcast (GPSIMD) -> mul;
          head-odd half shifted to partitions 64:128 via SBUF DMA
  y: per (Tk, 512-col) unit, 2 matmuls (K=256 over the head pairs) ->
          bf16 copy into a persistent per-chunk staging tile -> one
          512-row DMA per half chunk (large DMAs amortize the ~2.2us
          per-DMA completion serialization on the SP queue)

Emission / scheduling: the Tile scheduler is a readiness+priority list
scheduler; the per-engine streams it produces execute in order.  The
attention inner chain (S -> exp -> mask -> O) is ACT-bound, so the
emission interleaves next-chunk projection groups and y-projection
units into the attention stream as PE filler, paced by an ACT-vs-PE
deficit tracker (pay_debt).  The last chunk holds back a few y units
as tail warmers so the final normalize chain overlaps PE work.  All
DMAs are batched (one per weight matrix / x chunk / half y chunk) --
each DMA instruction costs ~650ns HWDGE issue plus ~1.5us completion
serialization on its queue.
"""
import numpy as np
import ml_dtypes

import concourse.tile as tile
import concourse.mybir as mybir
from concourse import bacc
from concourse.bass_utils import run_bass_kernel_spmd

FP = mybir.dt.float32
BF = mybir.dt.bfloat16
F8 = mybir.dt.float8e4
W8SCALE = 16.0  # fp8 weight pre-scale (folded back out via the exp scale)
NDT = 4  # DoubleRow K-tiles over the embedding dim (4 x (128x2))
B, T, C = 2, 2048, 1024
H, D = 16, 64
SCALE = 1.0 / 32.0  # 1/sqrt(C)
S_SCALE = SCALE / (W8SCALE * W8SCALE)  # q,k carry a 16x pre-scale each
N_CORES = 8
NKT = C // 128  # 8 K-tiles over the embedding dim
NTK = T // 128  # 16 Tk tiles
NI = T // 512  # 4 Tq chunks
EXP = mybir.ActivationFunctionType.Exp

_nc_cache = {}


def build_kernel(repeats=1, hmix=False):
    key = (repeats, hmix)
    if key in _nc_cache:
        return _nc_cache[key]

    nc = bacc.Bacc("TRN2", target_bir_lowering=False, debug=False)

    xT_d = nc.dram_tensor("xT", [C, T], BF, kind="ExternalInput").ap()
    xq8_d = nc.dram_tensor("xq8", [128, NDT, 2, T], F8, kind="ExternalInput").ap()
    wq8_d = nc.dram_tensor("wq8", [128, NDT, 2, 256], F8, kind="ExternalInput").ap()
    wk8_d = nc.dram_tensor("wk8", [128, NDT, 2, 256], F8, kind="ExternalInput").ap()
    wvT_d = nc.dram_tensor("wvT", [C, 256], BF, kind="ExternalInput").ap()
    woT_d = nc.dram_tensor("woT", [256, C], BF, kind="ExternalInput").ap()
    y_d = nc.dram_tensor("y", [T, C], BF, kind="ExternalOutput").ap()

    # mask[p, i] = 1 iff i >= p : triangular strip at the causal boundary
    mask_np = (
        np.arange(128)[None, :] >= np.arange(128)[:, None]
    ).astype(ml_dtypes.bfloat16)
    mask_d = nc.inline_tensor(mask_np, "mask_tri").ap()

    with tile.TileContext(nc) as tc:
        with (
            tc.tile_pool(name="persist", bufs=1) as pp,
            tc.tile_pool(name="ppool", bufs=4) as ppool,
            tc.tile_pool(name="spool", bufs=8) as spool,
            tc.tile_pool(name="ps_s", bufs=2, space="PSUM") as ps_s,
            tc.tile_pool(name="ps_o", bufs=2, space="PSUM") as ps_o,
            tc.tile_pool(name="ps_mm", bufs=2, space="PSUM") as ps_mm,
        ):
            # ---- critical-path DMAs first: the fp8 q/k operands are tiny,
            # so the first projection matmuls start in ~2-3us ----
            wq8 = pp.tile([128, NDT, 2, 256], F8, tag="wq8")
            nc.sync.dma_start(wq8[:, :, :, :], wq8_d[:, :, :, :])
            xq8 = [
                pp.tile([128, NDT, 2, 512], F8, tag=f"xq8{c}", name=f"xq8_{c}")
                for c in range(NI)
            ]
            nc.sync.dma_start(xq8[0][:, :, :, :], xq8_d[:, :, :, 0:512])
            wk8 = pp.tile([128, NDT, 2, 256], F8, tag="wk8")
            nc.scalar.dma_start(wk8[:, :, :, :], wk8_d[:, :, :, :])
            # xT (bf16) feeds only the V projection; one DMA per chunk
            xT_r = xT_d.rearrange("(n p) t -> p n t", p=128)
            xc = [
                pp.tile([128, NKT, 512], BF, tag=f"xc{c}", name=f"xc{c}")
                for c in range(NI)
            ]
            nc.sync.dma_start(xc[0][:, :, :], xT_r[:, :, 0:512])
            wv_big = pp.tile([128, NKT, 256], BF, tag="wv")
            nc.scalar.dma_start(
                wv_big[:, :, :], wvT_d.rearrange("(n p) d -> p n d", p=128)
            )
            for c in range(1, NI):
                nc.sync.dma_start(
                    xq8[c][:, :, :, :], xq8_d[:, :, :, c * 512 : c * 512 + 512]
                )
                nc.sync.dma_start(
                    xc[c][:, :, :], xT_r[:, :, c * 512 : c * 512 + 512]
                )
            wo_big = pp.tile([128, 2, C], BF, tag="wo")
            nc.scalar.dma_start(
                wo_big[:, :, :], woT_d.rearrange("(n p) d -> p n d", p=128)
            )
            mask = pp.tile([128, 128], BF, tag="mask")
            nc.scalar.dma_start(mask[:], mask_d[:])

            # preload the exp table while DMAs stream (saves ~1.3us later)
            warm_in = pp.tile([1, 2], FP, tag="warm_in")
            warm_out = pp.tile([1, 2], FP, tag="warm_out")
            nc.vector.memset(warm_in[:], 0.0)
            nc.scalar.activation(warm_out[:], warm_in[:], EXP, scale=SCALE)


            # ---- persistent activations ----
            qTc = [
                [pp.tile([128, 512], BF, tag=f"qT{p}_{i}", name=f"qT{p}_{i}") for i in range(NI)]
                for p in range(2)
            ]
            kTc = [
                [pp.tile([128, 512], BF, tag=f"kT{p}_{i}", name=f"kT{p}_{i}") for i in range(NI)]
                for p in range(2)
            ]
            otstc = [
                [pp.tile([128, 512], BF, tag=f"ot{p}_{i}", name=f"ot{p}_{i}") for i in range(NI)]
                for p in range(2)
            ]
            # vaug[t] = [Vh0|1|Vh1|1|Vh2|1|Vh3|1]; ones set once, V cols
            # rewritten per repeat by the strided copy from the V psum
            vaug = [pp.tile([128, 260], BF, tag=f"va{t}", name=f"va{t}") for t in range(NTK)]
            for t in range(NTK):
                nc.vector.memset(vaug[t][:, 64:260:65], 1.0)

            R = [0]

            # ---- PE filler machinery: queues of (cost_ns, emit_fn).
            # proj fillers must all land before the next attention chunk;
            # yproj fillers may linger until the final drain ----
            fillers_proj = []
            fillers_y = []
            debt = [0.0]
            y_reserve = [0]

            def pay_debt():
                while debt[0] > 200.0 and (
                    fillers_proj or len(fillers_y) > y_reserve[0]
                ):
                    q = fillers_proj if fillers_proj else fillers_y
                    cost, fn = q.pop(0)
                    fn()
                    debt[0] -= cost

            def drain_proj_fillers():
                while fillers_proj:
                    _, fn = fillers_proj.pop(0)
                    fn()
                debt[0] = 0.0

            def drain_y_fillers(on_act=False):
                while fillers_y:
                    _, fn = fillers_y.pop(0)
                    fn(on_act=on_act)

            def drain_all_fillers():
                drain_proj_fillers()
                drain_y_fillers(on_act="tail")

            # ---- emission units ----
            def emit_qk_group(c, wts, dsts, nm, pair):
                ps = ps_mm.tile([128, 512], FP, tag="mm",
                                name=f"ps{nm}{c}_{pair}_r{R[0]}")
                for dt in range(NDT):
                    nc.tensor.matmul(
                        ps[:],
                        lhsT=wts[:, dt, :, pair * 128 : pair * 128 + 128],
                        rhs=xq8[c][:, dt, :, :],
                        start=(dt == 0),
                        stop=(dt == NDT - 1),
                        perf_mode=mybir.MatmulPerfMode.DoubleRow,
                    )
                # this copy gates the next chunk's whole attention stream;
                # schedule it ahead of other queued DVE work
                with tc.high_priority(60):
                    nc.vector.tensor_copy(dsts[pair][c][:], ps[:])

            def emit_v_group(t):
                c, ts = t // 4, t % 4
                ps = ps_mm.tile([128, 512], FP, tag="mm",
                                name=f"psv{t}_r{R[0]}")
                for kk in range(NKT):
                    nc.tensor.matmul(
                        ps[:, 0:256],
                        lhsT=xc[c][:, kk, ts * 128 : ts * 128 + 128],
                        rhs=wv_big[:, kk, :],
                        start=(kk == 0),
                        stop=(kk == NKT - 1),
                    )
                with tc.high_priority(60):
                    nc.vector.tensor_copy(
                        vaug[t][:].rearrange("p (n d) -> p n d", n=4)[:, :, 0:64],
                        ps[:, 0:256].rearrange("p (n d) -> p n d", n=4),
                    )

            def emit_proj_chunk(c):
                for pair in range(2):
                    emit_qk_group(c, wq8, qTc, "q", pair)
                for pair in range(2):
                    emit_qk_group(c, wk8, kTc, "k", pair)
                for t in range(4 * c, 4 * c + 4):
                    emit_v_group(t)

            def proj_chunk_fillers(c):
                for pair in range(2):
                    fillers_proj.append((430, lambda pair=pair: emit_qk_group(
                        c, wq8, qTc, "q", pair)))
                for pair in range(2):
                    fillers_proj.append((430, lambda pair=pair: emit_qk_group(
                        c, wk8, kTc, "k", pair)))
                for t in range(4 * c, 4 * c + 4):
                    fillers_proj.append((870, lambda t=t: emit_v_group(t)))

            yt_chunk = [
                pp.tile([128, 4, 1024], BF, tag=f"yc{c}", name=f"yc{c}")
                for c in range(NI)
            ]
            y_done = {}

            def emit_yproj_unit(t, nch, on_act=False):
                ps = ps_mm.tile([128, 512], FP, tag="mm",
                                name=f"psy{t}_{nch}_r{R[0]}")
                for pair in range(2):
                    nc.tensor.matmul(
                        ps[:],
                        lhsT=otstc[pair][t // 4][
                            :, (t % 4) * 128 : (t % 4) * 128 + 128
                        ],
                        rhs=wo_big[:, pair, nch * 512 : nch * 512 + 512],
                        start=(pair == 0),
                        stop=(pair == 1),
                    )
                c = t // 4
                dst = yt_chunk[c][:, t % 4, nch * 512 : nch * 512 + 512]
                # mid-run copies go to DVE (ACT is exp-saturated); in the
                # reserve drain the DVE queue is full of normalize work so
                # use ACT; at the tail both are idle, so alternate
                if on_act == "act" or (
                    on_act == "tail" and (2 * t + nch) % 2 == 0
                ):
                    nc.scalar.copy(dst, ps[:])
                else:
                    nc.vector.tensor_copy(dst, ps[:])
                y_done[c] = y_done.get(c, 0) + 1
                if y_done[c] in (2, 4, 6, 8):  # quarter-chunk -> DMA it
                    lo = y_done[c] // 2 - 1
                    nc.sync.dma_start(
                        y_d[c * 512 + lo * 128 : c * 512 + lo * 128 + 128, :]
                        .rearrange("(n p) d -> p n d", p=128),
                        yt_chunk[c][:, lo : lo + 1, :],
                    )
                    if y_done[c] == 8:
                        y_done[c] = 0

            def emit_attention(I, last=False, pairs=(0, 1)):
                emit_attention_body(I, last, pairs)
                if 1 in pairs:
                    for t in range(4 * I, 4 * I + 4):
                        for nch in range(2):
                            fillers_y.append(
                                (430,
                                 lambda t=t, nch=nch, **kw: emit_yproj_unit(
                                     t, nch, **kw))
                            )

            def emit_attention_body(I, last, pairs):
                jmax = 4 * I + 4
                for pair in pairs:
                    oT = [
                        ps_o.tile([65, 512], FP, tag="oT",
                                  name=f"o{I}_{pair}_{h}_r{R[0]}")
                        for h in range(2)
                    ]

                    def emit_o(g, tiles):
                        j0 = 2 * g
                        for h in range(2):
                            p_sb = tiles[h]
                            head = 2 * pair + h
                            for dj in range(2):
                                j = j0 + dj
                                z = max(0, j * 128 - I * 512)
                                nc.tensor.matmul(
                                    oT[h][:, z:512],
                                    lhsT=vaug[j][:, 65 * head : 65 * head + 65],
                                    rhs=p_sb[:, dj * 512 + z : dj * 512 + 512],
                                    start=(j == 0),
                                    stop=(j == jmax - 1),
                                )

                    prev = None
                    for g in range(jmax // 2):
                        j0 = 2 * g
                        diag = j0 >= 4 * I
                        zs = [max(0, (j0 + dj) * 128 - I * 512) for dj in range(2)]
                        cur = []
                        act_ns = 0.0
                        cols = 0
                        for h in range(2):
                            hsl = slice(64 * h, 64 * h + 64)
                            s_ps = ps_s.tile([128, 1024], FP, tag="s",
                                             name=f"s{I}_{pair}_{h}_{g}_r{R[0]}")
                            for dj in range(2):
                                j = j0 + dj
                                z = zs[dj]
                                nc.tensor.matmul(
                                    s_ps[:, dj * 512 + z : dj * 512 + 512],
                                    lhsT=kTc[pair][j // 4][
                                        hsl, (j % 4) * 128 : (j % 4) * 128 + 128
                                    ],
                                    rhs=qTc[pair][I][hsl, z:512],
                                    start=True,
                                    stop=True,
                                )
                                cols += 512 - z
                            p_sb = ppool.tile([128, 1024], BF, tag="p",
                                              name=f"p{I}_{pair}_{h}_{g}_r{R[0]}")
                            if not diag or zs[0] == 0:
                                # diag group with z0=0: one act over the whole
                                # tile; the uncomputed gap [512:512+z1] holds
                                # stale S values, bounded so exp stays finite,
                                # and nothing downstream reads it
                                nc.scalar.activation(p_sb[:], s_ps[:], EXP,
                                                     scale=S_SCALE)
                                act_ns += 1024 * 0.833 + 185
                            else:
                                for dj in range(2):
                                    lo = dj * 512 + zs[dj]
                                    hi = dj * 512 + 512
                                    nc.scalar.activation(
                                        p_sb[:, lo:hi], s_ps[:, lo:hi], EXP,
                                        scale=S_SCALE,
                                    )
                                    act_ns += (hi - lo) * 0.833 + 185
                            for dj in range(2):
                                j = j0 + dj
                                if j >= 4 * I:
                                    z = zs[dj]
                                    ssl = slice(dj * 512 + z, dj * 512 + z + 128)
                                    nc.vector.tensor_mul(
                                        p_sb[:, ssl], p_sb[:, ssl], mask[:]
                                    )
                            cur.append(p_sb)
                        # ACT-vs-PE deficit for this group: exp time vs the
                        # S+O matmul time (2x cols at 0.4167ns/col)
                        debt[0] += act_ns - 2 * cols * 0.4167
                        if prev is not None:
                            emit_o(g - 1, prev)
                        pay_debt()
                        prev = cur
                    emit_o(jmax // 2 - 1, prev)
                    if last and pair == 1:
                        # reserved y units keep PE warm through the final
                        # normalize chain
                        y_reserve[0] = 0
                        drain_y_fillers(on_act="act")
                    # normalize O^T by the PSUM row-64 denominator
                    for h in range(2):
                        recip = spool.tile([1, 512], FP, tag="recip",
                                           name=f"rc{I}_{pair}_{h}_r{R[0]}")
                        nc.vector.reciprocal(recip[:], oT[h][64:65, :])
                        bcast = spool.tile([64, 512], FP, tag="bcast",
                                           name=f"bc{I}_{pair}_{h}_r{R[0]}")
                        nc.gpsimd.partition_broadcast(bcast[:], recip[:])
                        if h == 0:
                            nc.vector.tensor_mul(
                                otstc[pair][I][0:64, :], oT[h][0:64, :], bcast[:]
                            )
                        else:
                            onrm = spool.tile([64, 512], BF, tag="onrm",
                                              name=f"on{I}_{pair}_r{R[0]}")
                            nc.vector.tensor_mul(onrm[:], oT[h][0:64, :], bcast[:])
                            # partition shift 0->64 needs a DMA
                            nc.sync.dma_start(otstc[pair][I][64:128, :], onrm[:])


            # ---- main emission.  Attention instructions carry high
            # scheduler priority (they form the serial latency chain:
            # S -> exp -> mask -> O -> normalize); projections and
            # y-projection units are normal priority, so the greedy
            # scheduler slots them into PE whenever attention work is
            # not ready ----
            for rep in range(repeats):
                R[0] = rep
                emit_proj_chunk(0)
                for c in range(NI):
                    if c + 1 < NI:
                        proj_chunk_fillers(c + 1)
                    else:
                        y_reserve[0] = 6
                    emit_attention(c, last=(c == NI - 1))
                    drain_proj_fillers()
                y_reserve[0] = 0
                drain_all_fillers()

    nc.compile()
    _nc_cache[key] = nc
    return nc


def make_in_maps(x, Wq, Wk, Wv, Wo):
    x = np.asarray(x, dtype=np.float32)
    Wq = np.asarray(Wq, dtype=np.float32)
    Wk = np.asarray(Wk, dtype=np.float32)
    Wv = np.asarray(Wv, dtype=np.float32)
    Wo = np.asarray(Wo, dtype=np.float32)
    bf = ml_dtypes.bfloat16
    f8 = ml_dtypes.float8_e4m3fn

    def dr_pack(a):  # [C, m] -> [128, NDT, 2, m] with k = 256*dt + ki + 128*ko
        return np.ascontiguousarray(
            a.reshape(4, 2, 128, -1).transpose(2, 0, 1, 3)
        )

    in_maps = []
    for c in range(N_CORES):
        b, hg = c // 4, c % 4
        sl = slice(256 * hg, 256 * hg + 256)
        xTb = x[b].T
        in_maps.append(
            {
                "xT": np.ascontiguousarray(xTb).astype(bf),
                "xq8": dr_pack(xTb.astype(f8)),
                "wq8": dr_pack((Wq[sl, :].T * W8SCALE).astype(f8)),
                "wk8": dr_pack((Wk[sl, :].T * W8SCALE).astype(f8)),
                "wvT": np.ascontiguousarray(Wv[sl, :].T).astype(bf),
                "woT": np.ascontiguousarray(Wo[:, sl].T).astype(bf),
            }
        )
    return in_maps


def run_spmd(in_maps, trace=False, repeats=1, **kw):
    nc = build_kernel(repeats)
    return run_bass_kernel_spmd(nc, in_maps, list(range(N_CORES)), trace=trace, **kw)


def gather(results, bo):
    bo = np.asarray(bo, dtype=np.float32)
    y = np.empty((B, T, C), dtype=np.float32)
    for b in range(B):
        acc = results[4 * b]["y"].astype(np.float32).copy()
        for g in range(1, 4):
            acc += results[4 * b + g]["y"].astype(np.float32)
        y[b] = acc + bo[None, :]
    return y


def kernel(x, Wq, Wk, Wv, Wo, bo):
    res = run_spmd(make_in_maps(x, Wq, Wk, Wv, Wo))
    return gather(res.results, bo)
